# revision 10
# baseline (speedup 1.0000x reference)
# Trainium2 Bass kernel for the 2-layer GNN message-passing block.
# Self-contained: hardcodes shapes; takes full inputs, shards across 8 cores,
# returns the full [50000, 128] float32 output.
import os
import sys

sys.path.insert(0, "/opt/trn_rl_repo")

import numpy as np
import ml_dtypes

import concourse.bacc as bacc
import concourse.tile as tile
from concourse import mybir
from concourse.bass_utils import run_bass_kernel_spmd

BF16 = ml_dtypes.bfloat16

N = 50000
NPAD = 50176
NC = 8
C = NPAD // NC            # 6272 nodes per core
WCNT = C // 128           # 49 windows of 128 nodes
HALF = NPAD // 2          # 25088 (int16-addressable gather halves)
GOP = int(os.environ.get("KERNEL_GOP", "1024"))  # edges per dma_gather op
SBC = 8                   # chunks (of 128 edges) per compute sub-batch

F32 = mybir.dt.float32
BF = mybir.dt.bfloat16
I16 = mybir.dt.int16


def _bf(x):
    return np.ascontiguousarray(x.astype(BF16))


def _prep(inputs):
    """Host-side graph partitioning. Returns per-core input dicts + metadata."""
    src = np.asarray(inputs["edge_index"][0]).astype(np.int64)
    dst = np.asarray(inputs["edge_index"][1]).astype(np.int64)
    ef = np.asarray(inputs["edge_features"]).astype(np.float32)
    E = src.shape[0]

    owner = dst // C
    halfe = src // HALF
    dl = dst - owner * C
    win = dl // 128
    wl = dl % 128

    key = (owner * 2 + halfe) * WCNT + win
    order = np.argsort(key, kind="stable")
    ksort = key[order]
    counts_flat = np.bincount(key, minlength=NC * 2 * WCNT)
    counts = counts_flat.reshape(NC, 2, WCNT)

    nch = np.maximum(1, -(-counts.max(axis=0) // 128)).astype(np.int64)  # [2,WCNT]
    seg_len = nch * 128
    L0s = int(seg_len[0].sum())
    L1s = int(seg_len[1].sum())
    pad0 = (-L0s) % GOP
    L0p = L0s + pad0
    pad1 = (-L1s) % GOP
    L = L0p + L1s + pad1

    seg_start = np.zeros((2, WCNT), np.int64)
    pos = 0
    for w in range(WCNT):
        seg_start[0, w] = pos
        pos += seg_len[0, w]
    pos = L0p
    for w in range(WCNT):
        seg_start[1, w] = pos
        pos += seg_len[1, w]

    # destination position of each edge within its core's stream
    group_first = np.cumsum(counts_flat) - counts_flat
    within = np.arange(E, dtype=np.int64) - group_first[ksort]
    dest = seg_start[halfe[order], win[order]] + within

    owner_s = owner[order]

    # chunk metadata: (window, is_first, is_last, real?) per 128-edge chunk
    nchunks = L // 128
    chunk_meta = []
    cw = np.full(nchunks, -1, np.int64)
    cf = np.zeros(nchunks, bool)
    cl = np.zeros(nchunks, bool)
    ch = np.zeros(nchunks, np.int64)
    for h in range(2):
        for w in range(WCNT):
            s = int(seg_start[h, w]) // 128
            n = int(nch[h, w])
            cw[s:s + n] = w
            cf[s] = True
            cl[s + n - 1] = True
            ch[s:s + n] = h
    for cc in range(nchunks):
        chunk_meta.append((int(cw[cc]), bool(cf[cc]), bool(cl[cc]), int(ch[cc]),
                           cw[cc] >= 0))

    x = np.asarray(inputs["x"]).astype(np.float32)
    xpad = np.zeros((NPAD, 128), np.float32)
    xpad[:N] = x
    xT_bf = _bf(xpad.T)

    iota = np.tile(np.arange(128, dtype=np.float32)[None, :], (128, 1))
    ident = np.eye(128, dtype=np.float32)

    shared = {
        "xT": xT_bf,
        "W1": _bf(np.asarray(inputs["ff1_W"], np.float32)),
        "Ws1": _bf(np.asarray(inputs["mp1_Wsrc"], np.float32)),
        "Wd1": _bf(np.asarray(inputs["mp1_Wdst"], np.float32)),
        "We1": _bf(np.asarray(inputs["mp1_We"], np.float32)),
        "Ws2": _bf(np.asarray(inputs["mp2_Wsrc"], np.float32)),
        "Wd2": _bf(np.asarray(inputs["mp2_Wdst"], np.float32)),
        "We2": _bf(np.asarray(inputs["mp2_We"], np.float32)),
        "W3": _bf(np.asarray(inputs["ff2_W"], np.float32)),
        "b1c": np.ascontiguousarray(
            np.asarray(inputs["ff1_b"], np.float32)[:, None]),
        "b1m": np.ascontiguousarray(
            np.tile(np.asarray(inputs["mp1_b"], np.float32)[None, :], (128, 1))),
        "b2m": np.ascontiguousarray(
            np.tile(np.asarray(inputs["mp2_b"], np.float32)[None, :], (128, 1))),
        "b3m": np.ascontiguousarray(
            np.tile(np.asarray(inputs["ff2_b"], np.float32)[None, :], (128, 1))),
        "iota": _bf(iota),
        "identb": _bf(ident),
        "identf": ident,
    }

    per_core = []
    for c in range(NC):
        m = owner_s == c
        e_ids = order[m]
        dp = dest[m]
        eT = np.zeros((64, L), np.float32)
        eT[:, dp] = ef[e_ids].T
        edc = np.full(L, -1.0, np.float32)
        edc[dp] = wl[e_ids]
        sidx = np.zeros(L, np.int16)
        sidx[dp] = (src[e_ids] - halfe[e_ids] * HALF).astype(np.int16)
        didx = np.zeros(L, np.int16)
        didx[dp] = (dst[e_ids] - c * C).astype(np.int16)

        # wrap layouts
        edc_cw = np.ascontiguousarray(edc.reshape(L // 128, 128).T)     # [128, L/128]
        s_wr = np.ascontiguousarray(
            np.tile(sidx.reshape(L // 16, 16).T, (8, 1)))               # [128, L/16]
        d_wr = np.ascontiguousarray(
            np.tile(didx.reshape(L // 16, 16).T, (8, 1)))               # [128, L/16]

        per_core.append({
            "eT": _bf(eT),
            "edc": _bf(edc_cw),
            "srcw": s_wr,
            "xdw": d_wr,
            "xoT": _bf(xpad[c * C:(c + 1) * C].T),
        })
    meta = dict(L=L, L0p=L0p, chunk_meta=chunk_meta)
    return shared, per_core, meta


def _build(L, L0p, chunk_meta):
    """Build the SPMD Bass program (identical for all 8 cores)."""
    nc = bacc.Bacc("TRN2", target_bir_lowering=False, debug=False, num_devices=NC,
                   dynamic_dma_scratch_size=int(os.environ.get("KERNEL_DDS", "16384")))
    GELU = (mybir.ActivationFunctionType.Identity
            if os.environ.get("KERNEL_SIM_IDENTITY") == "1"
            else mybir.ActivationFunctionType.Gelu_apprx_tanh)
    EQ = mybir.AluOpType.is_equal

    # I/O
    t_xT = nc.dram_tensor("xT", [128, NPAD], BF, kind="ExternalInput")
    t_xoT = nc.dram_tensor("xoT", [128, C], BF, kind="ExternalInput")
    t_eT = nc.dram_tensor("eT", [64, L], BF, kind="ExternalInput")
    t_edc = nc.dram_tensor("edc", [128, L // 128], BF, kind="ExternalInput")
    t_srcw = nc.dram_tensor("srcw", [128, L // 16], I16, kind="ExternalInput")
    t_xdw = nc.dram_tensor("xdw", [128, L // 16], I16, kind="ExternalInput")
    wts = {}
    for nm, shape, dt in [
        ("W1", [128, 128], BF), ("Ws1", [128, 128], BF), ("Wd1", [128, 128], BF),
        ("We1", [64, 128], BF), ("Ws2", [128, 128], BF), ("Wd2", [128, 128], BF),
        ("We2", [64, 128], BF), ("W3", [128, 128], BF),
        ("b1c", [128, 1], F32), ("b1m", [128, 128], F32), ("b2m", [128, 128], F32),
        ("b3m", [128, 128], F32), ("iota", [128, 128], BF),
        ("identb", [128, 128], BF), ("identf", [128, 128], F32),
    ]:
        wts[nm] = nc.dram_tensor(nm, shape, dt, kind="ExternalInput")
    t_out = nc.dram_tensor("out", [C, 128], F32, kind="ExternalOutput")

    NOPS = L // GOP
    NCH = L // 128

    with tile.TileContext(nc) as tc:
        with (
            tc.tile_pool(name="persist", bufs=1) as pp,
            tc.tile_pool(name="dram", bufs=1, space="DRAM") as dram,
        ):
            # persistent SBUF state
            wt = {}
            for nm in ["W1", "Ws1", "Wd1", "We1", "Ws2", "Wd2", "We2", "W3",
                       "b1c", "b1m", "b2m", "b3m", "iota", "identb", "identf"]:
                shape = wts[nm].shape
                dt = {"b1c": F32, "b1m": F32, "b2m": F32, "b3m": F32,
                      "identf": F32}.get(nm, BF)
                wt[nm] = pp.tile(list(shape), dt, tag=f"w_{nm}", name=f"w_{nm}")
                nc.sync.dma_start(out=wt[nm][:], in_=wts[nm][:])
            edc_t = pp.tile([128, NCH], BF, tag="edc")
            nc.sync.dma_start(out=edc_t[:], in_=t_edc[:])
            srcw_t = pp.tile([128, L // 16], I16, tag="srcw")
            nc.sync.dma_start(out=srcw_t[:], in_=t_srcw[:])
            xdw_t = pp.tile([128, L // 16], I16, tag="xdw")
            nc.sync.dma_start(out=xdw_t[:], in_=t_xdw[:])
            h_own = pp.tile([128, C], BF, tag="h_own")      # node-major own windows
            agg_sb = pp.tile([128, C], F32, tag="agg_sb")   # per-window agg (half 0)

            # internal DRAM
            xs_d = [dram.tile([NPAD, 128], BF, tag=f"xs{l}", name=f"xs{l}") for l in range(2)]
            xd_d = [dram.tile([C, 128], BF, tag=f"xd{l}", name=f"xd{l}") for l in range(2)]
            ag_in = dram.tile([C, 128], BF, tag="ag_in")
            ag_out = dram.tile([NPAD, 128], BF, tag="ag_out", addr_space="Shared")

            r4096 = nc.gpsimd.to_reg(GOP)

            # ---------------- dense phase ----------------
            def dense_full(layer):
                """xs[layer] for all NPAD nodes."""
                Ws = wt["Ws1"] if layer == 0 else wt["Ws2"]
                with (
                    tc.tile_pool(name=f"dA{layer}", bufs=3) as dp,
                    tc.tile_pool(name=f"dAp{layer}", bufs=2, space="PSUM") as dq,
                ):
                    for g in range(NPAD // 512):
                        hT = dp.tile([128, 512], BF, tag="hT")
                        if layer == 0:
                            xt = dp.tile([128, 512], BF, tag="xt")
                            nc.sync.dma_start(
                                out=xt[:], in_=t_xT[:, g * 512:(g + 1) * 512])
                            ps = dq.tile([128, 512], F32, tag="ps")
                            nc.tensor.matmul(out=ps[:], lhsT=wt["W1"][:],
                                             rhs=xt[:], start=True, stop=True)
                            nc.scalar.activation(out=hT[:], in_=ps[:], func=GELU,
                                                 bias=wt["b1c"][:])
                        else:
                            nc.sync.dma_start(
                                out=hT[:],
                                in_=ag_out[g * 512:(g + 1) * 512, :],
                                transpose=True)
                        for j in range(4):
                            n0 = g * 512 + j * 128
                            px = dq.tile([128, 128], F32, tag="px")
                            nc.tensor.matmul(out=px[:],
                                             lhsT=hT[:, j * 128:(j + 1) * 128],
                                             rhs=Ws[:], start=True, stop=True)
                            xs_sb = dp.tile([128, 128], BF, tag="xs_sb")
                            nc.vector.tensor_copy(out=xs_sb[:], in_=px[:])
                            nc.sync.dma_start(out=xs_d[layer][n0:n0 + 128, :],
                                              in_=xs_sb[:])

            def dense_own_l0():
                """h0 own (node-major) + xd0+b for own nodes, from x_own_T."""
                with (
                    tc.tile_pool(name="dB", bufs=3) as dp,
                    tc.tile_pool(name="dBp", bufs=2, space="PSUM") as dq,
                ):
                    ngrp = (C + 511) // 512
                    for g in range(ngrp):
                        c0 = g * 512
                        cn = min(512, C - c0)
                        xt = dp.tile([128, 512], BF, tag="xt")
                        nc.sync.dma_start(out=xt[:, :cn], in_=t_xoT[:, c0:c0 + cn])
                        ps = dq.tile([128, 512], F32, tag="ps")
                        nc.tensor.matmul(out=ps[:, :cn], lhsT=wt["W1"][:],
                                         rhs=xt[:, :cn], start=True, stop=True)
                        hT = dp.tile([128, 512], BF, tag="hT")
                        nc.scalar.activation(out=hT[:, :cn], in_=ps[:, :cn],
                                             func=GELU, bias=wt["b1c"][:])
                        for j in range(cn // 128):
                            lw = c0 + j * 128
                            sl = hT[:, j * 128:(j + 1) * 128]
                            pn = dq.tile([128, 128], F32, tag="pn")
                            nc.tensor.matmul(out=pn[:], lhsT=sl, rhs=wt["identb"][:],
                                             start=True, stop=True)
                            nc.vector.tensor_copy(out=h_own[:, lw:lw + 128], in_=pn[:])
                            pd = dq.tile([128, 128], F32, tag="pd")
                            nc.tensor.matmul(out=pd[:], lhsT=sl, rhs=wt["Wd1"][:],
                                             start=True, stop=True)
                            xd_sb = dp.tile([128, 128], BF, tag="xd_sb")
                            nc.vector.tensor_add(out=xd_sb[:], in0=pd[:],
                                                 in1=wt["b1m"][:])
                            nc.sync.dma_start(out=xd_d[0][lw:lw + 128, :],
                                              in_=xd_sb[:])

            def dense_own_l1():
                """xd1+b for own nodes from h_own (h1, node-major in SBUF)."""
                with (
                    tc.tile_pool(name="dC", bufs=3) as dp,
                    tc.tile_pool(name="dCp", bufs=2, space="PSUM") as dq,
                ):
                    for w in range(WCNT):
                        ws = slice(w * 128, (w + 1) * 128)
                        pt = dq.tile([128, 128], BF, tag="pt")
                        nc.tensor.transpose(out=pt[:], in_=h_own[:, ws],
                                            identity=wt["identb"][:])
                        hT = dp.tile([128, 128], BF, tag="hT")
                        nc.vector.tensor_copy(out=hT[:], in_=pt[:])
                        pd = dq.tile([128, 128], F32, tag="pd")
                        nc.tensor.matmul(out=pd[:], lhsT=hT[:], rhs=wt["Wd2"][:],
                                         start=True, stop=True)
                        xd_sb = dp.tile([128, 128], BF, tag="xd_sb")
                        nc.vector.tensor_add(out=xd_sb[:], in0=pd[:], in1=wt["b2m"][:])
                        nc.sync.dma_start(out=xd_d[1][w * 128:(w + 1) * 128, :],
                                          in_=xd_sb[:])

            # ---------------- edge phase ----------------
            def edge_phase(layer):
                We = wt["We1"] if layer == 0 else wt["We2"]
                with (
                    tc.tile_pool(name=f"eS{layer}", bufs=2) as ep,
                    tc.tile_pool(name=f"eG{layer}", bufs=2) as gp,
                    tc.tile_pool(name=f"eP{layer}", bufs=2, space="PSUM") as qp,
                    tc.tile_pool(name=f"eA{layer}", bufs=2, space="PSUM") as ap_,
                    tc.tile_pool(name=f"eF{layer}", bufs=2) as fp,
                ):
                    active_agg = [None]

                    def finalize(h, w, agg_ps):
                        ws = slice(w * 128, (w + 1) * 128)
                        if h == 0:
                            nc.vector.tensor_copy(out=agg_sb[:, ws], in_=agg_ps[:])
                            return
                        t1 = fp.tile([128, 128], F32, tag="f1")
                        nc.vector.tensor_add(out=t1[:], in0=agg_ps[:],
                                             in1=agg_sb[:, ws])
                        if layer == 0:
                            h1w = fp.tile([128, 128], BF, tag="fh")
                            nc.vector.tensor_add(out=h1w[:], in0=t1[:],
                                                 in1=h_own[:, ws])
                            nc.vector.tensor_copy(out=h_own[:, ws], in_=h1w[:])
                            nc.sync.dma_start(out=ag_in[w * 128:(w + 1) * 128, :],
                                              in_=h1w[:])
                        else:
                            h2w = fp.tile([128, 128], F32, tag="fh2")
                            nc.vector.tensor_add(out=h2w[:], in0=t1[:],
                                                 in1=h_own[:, ws])
                            pt = ap_.tile([128, 128], F32, tag="tp", bufs=1)
                            nc.tensor.transpose(out=pt[:], in_=h2w[:],
                                                identity=wt["identf"][:])
                            h2T = fp.tile([128, 128], BF, tag="h2T")
                            nc.vector.tensor_copy(out=h2T[:], in_=pt[:])
                            po = ap_.tile([128, 128], F32, tag="po", bufs=1)
                            nc.tensor.matmul(out=po[:], lhsT=h2T[:], rhs=wt["W3"][:],
                                             start=True, stop=True)
                            osb = fp.tile([128, 128], F32, tag="osb")
                            nc.vector.tensor_add(out=osb[:], in0=po[:],
                                                 in1=wt["b3m"][:])
                            nc.sync.dma_start(out=t_out[w * 128:(w + 1) * 128, :],
                                              in_=osb[:])

                    for op_i in range(NOPS):
                        e0 = op_i * GOP
                        cc0 = e0 // 128
                        nreal = sum(1 for k in range(GOP // 128) if chunk_meta[cc0 + k][4])
                        if nreal == 0:
                            continue
                        half = 0 if e0 < L0p else 1
                        src_ap = xs_d[layer][half * HALF:(half + 1) * HALF, :]
                        xsg = gp.tile([128, GOP // 128, 128], BF, tag="xsg")
                        nc.gpsimd.dma_gather(
                            xsg[:], src_ap, srcw_t[:, e0 // 16:(e0 + GOP) // 16],
                            GOP, r4096, 128, elem_step=128)
                        xdg = gp.tile([128, GOP // 128, 128], BF, tag="xdg")
                        nc.gpsimd.dma_gather(
                            xdg[:], xd_d[layer][:], xdw_t[:, e0 // 16:(e0 + GOP) // 16],
                            GOP, r4096, 128, elem_step=128)
                        eTt = ep.tile([64, GOP], BF, tag="eTt")
                        nc.sync.dma_start(out=eTt[:], in_=t_eT[:, e0:e0 + GOP])

                        for sb0 in range(0, nreal, SBC):
                            nb = min(SBC, nreal - sb0)
                            psq = qp.tile([128, SBC, 128], F32, tag="psq")
                            for j in range(nb):
                                cl = sb0 + j
                                nc.tensor.matmul(
                                    out=psq[:, j, :],
                                    lhsT=eTt[:, cl * 128:(cl + 1) * 128],
                                    rhs=We[:], start=True, stop=True)
                            tsx = ep.tile([128, SBC, 128], BF, tag="tsx")
                            nc.vector.tensor_add(out=tsx[:, :nb, :],
                                                 in0=xsg[:, sb0:sb0 + nb, :],
                                                 in1=xdg[:, sb0:sb0 + nb, :])
                            qsb = ep.tile([128, SBC, 128], F32, tag="qsb")
                            nc.vector.tensor_add(out=qsb[:, :nb, :],
                                                 in0=psq[:, :nb, :],
                                                 in1=tsx[:, :nb, :])
                            mt = ep.tile([128, SBC, 128], BF, tag="mt")
                            nc.scalar.activation(out=mt[:, :nb, :],
                                                 in_=qsb[:, :nb, :], func=GELU)
                            P4 = ep.tile([128, SBC, 128], BF, tag="P4")
                            nc.vector.tensor_tensor(
                                out=P4[:, :nb, :],
                                in0=edc_t[:, cc0 + sb0:cc0 + sb0 + nb]
                                    .to_broadcast([128, nb, 128]),
                                in1=wt["iota"][:, None, :]
                                    .to_broadcast([128, nb, 128]),
                                op=EQ)
                            for j in range(nb):
                                cc = cc0 + sb0 + j
                                w, first, last, hh, real = chunk_meta[cc]
                                assert real
                                if first:
                                    active_agg[0] = ap_.tile([128, 128], F32, tag="agg", name="agg_ps")
                                nc.tensor.matmul(out=active_agg[0][:],
                                                 lhsT=P4[:, j, :], rhs=mt[:, j, :],
                                                 start=first, stop=last)
                                if last:
                                    finalize(hh, w, active_agg[0])

            # ---------------- program ----------------
            phases = int(os.environ.get("KERNEL_PHASES", "5"))
            dense_full(0)
            dense_own_l0()
            if phases >= 2:
                edge_phase(0)
            if phases >= 3:
                nc.gpsimd.collective_compute(
                    "AllGather", mybir.AluOpType.bypass,
                    replica_groups=[list(range(NC))],
                    ins=[ag_in[:].opt()], outs=[ag_out[:].opt()])
            if phases >= 4:
                dense_full(1)
                dense_own_l1()
            if phases >= 5:
                edge_phase(1)
            else:
                with tc.tile_pool(name="dbg", bufs=2) as dbp:
                    for w in range(WCNT):
                        dsb = dbp.tile([128, 128], F32, tag="dsb")
                        nc.vector.tensor_copy(out=dsb[:], in_=h_own[:, w * 128:(w + 1) * 128])
                        nc.sync.dma_start(out=t_out[w * 128:(w + 1) * 128, :], in_=dsb[:])

    nc.finalize()
    return nc


_CACHE = {}


def _get_program(L, L0p, chunk_meta):
    key = (L, L0p, tuple(m[:4] for m in chunk_meta))
    if key not in _CACHE:
        _CACHE[key] = _build(L, L0p, chunk_meta)
    return _CACHE[key]


def kernel(**inputs):
    shared, per_core, meta = _prep(inputs)
    nc = _get_program(meta["L"], meta["L0p"], meta["chunk_meta"])
    in_maps = []
    for c in range(NC):
        m = dict(shared)
        m.update(per_core[c])
        in_maps.append(m)
    trace = os.environ.get("KERNEL_TRACE", "0") == "1"
    kw = {}
    if trace:
        kw = dict(trace=True, trace_kwargs={"title": "gnn_mp"})
    res = run_bass_kernel_spmd(nc, in_maps, core_ids=list(range(NC)), **kw)
    if trace and res.exec_time_ns is not None:
        print(f"HW exec time: {res.exec_time_ns} ns")
        if res.instructions_and_trace:
            print("trace:", res.instructions_and_trace[1])
    out = np.concatenate([res.results[c]["out"] for c in range(NC)], axis=0)
    return np.ascontiguousarray(out[:N]).astype(np.float32)


# revision 12
# speedup vs baseline: 1.6631x; 1.6631x over previous
# Trainium2 Bass kernel for the 2-layer GNN message-passing block.
# Self-contained: hardcodes shapes; takes full inputs, shards across 8 cores,
# returns the full [50000, 128] float32 output.
import os
import sys

sys.path.insert(0, "/opt/trn_rl_repo")

import numpy as np
import ml_dtypes

import concourse.bacc as bacc
import concourse.tile as tile
from concourse import mybir
from concourse.bass_utils import run_bass_kernel_spmd

BF16 = ml_dtypes.bfloat16

N = 50000
NPAD = 50176
NC = 8
C = NPAD // NC            # 6272 nodes per core
WCNT = C // 128           # 49 windows of 128 nodes
HALF = NPAD // 2          # 25088 (int16-addressable gather halves)
GOP = int(os.environ.get("KERNEL_GOP", "1024"))  # edges per dma_gather op
SBC = 8                   # chunks (of 128 edges) per compute sub-batch

F32 = mybir.dt.float32
BF = mybir.dt.bfloat16
I16 = mybir.dt.int16


def _bf(x):
    return np.ascontiguousarray(x.astype(BF16))


def _prep(inputs):
    """Host-side graph partitioning. Returns per-core input dicts + metadata."""
    src = np.asarray(inputs["edge_index"][0]).astype(np.int64)
    dst = np.asarray(inputs["edge_index"][1]).astype(np.int64)
    ef = np.asarray(inputs["edge_features"]).astype(np.float32)
    E = src.shape[0]

    owner = dst // C
    halfe = src // HALF
    dl = dst - owner * C
    win = dl // 128
    wl = dl % 128

    key = (owner * 2 + halfe) * WCNT + win
    order = np.argsort(key, kind="stable")
    ksort = key[order]
    counts_flat = np.bincount(key, minlength=NC * 2 * WCNT)
    counts = counts_flat.reshape(NC, 2, WCNT)

    nch = np.maximum(1, -(-counts.max(axis=0) // 128)).astype(np.int64)  # [2,WCNT]
    seg_len = nch * 128
    L0s = int(seg_len[0].sum())
    L1s = int(seg_len[1].sum())
    pad0 = (-L0s) % GOP
    L0p = L0s + pad0
    pad1 = (-L1s) % GOP
    L = L0p + L1s + pad1

    seg_start = np.zeros((2, WCNT), np.int64)
    pos = 0
    for w in range(WCNT):
        seg_start[0, w] = pos
        pos += seg_len[0, w]
    pos = L0p
    for w in range(WCNT):
        seg_start[1, w] = pos
        pos += seg_len[1, w]

    # destination position of each edge within its core's stream
    group_first = np.cumsum(counts_flat) - counts_flat
    within = np.arange(E, dtype=np.int64) - group_first[ksort]
    dest = seg_start[halfe[order], win[order]] + within

    owner_s = owner[order]

    # chunk metadata: (window, is_first, is_last, real?) per 128-edge chunk
    nchunks = L // 128
    chunk_meta = []
    cw = np.full(nchunks, -1, np.int64)
    cf = np.zeros(nchunks, bool)
    cl = np.zeros(nchunks, bool)
    ch = np.zeros(nchunks, np.int64)
    for h in range(2):
        for w in range(WCNT):
            s = int(seg_start[h, w]) // 128
            n = int(nch[h, w])
            cw[s:s + n] = w
            cf[s] = True
            cl[s + n - 1] = True
            ch[s:s + n] = h
    for cc in range(nchunks):
        chunk_meta.append((int(cw[cc]), bool(cf[cc]), bool(cl[cc]), int(ch[cc]),
                           cw[cc] >= 0))

    x = np.asarray(inputs["x"]).astype(np.float32)
    xpad = np.zeros((NPAD, 128), np.float32)
    xpad[:N] = x
    xT_bf = _bf(xpad.T)

    iota = np.tile(np.arange(128, dtype=np.float32)[None, :], (128, 1))
    ident = np.eye(128, dtype=np.float32)

    shared = {
        "xT": xT_bf,
        "W1": _bf(np.asarray(inputs["ff1_W"], np.float32)),
        "Ws1": _bf(np.asarray(inputs["mp1_Wsrc"], np.float32)),
        "Wd1": _bf(np.asarray(inputs["mp1_Wdst"], np.float32)),
        "We1": _bf(np.asarray(inputs["mp1_We"], np.float32)),
        "Ws2": _bf(np.asarray(inputs["mp2_Wsrc"], np.float32)),
        "Wd2": _bf(np.asarray(inputs["mp2_Wdst"], np.float32)),
        "We2": _bf(np.asarray(inputs["mp2_We"], np.float32)),
        "W3": _bf(np.asarray(inputs["ff2_W"], np.float32)),
        "b1c": np.ascontiguousarray(
            np.asarray(inputs["ff1_b"], np.float32)[:, None]),
        "b1m": np.ascontiguousarray(
            np.tile(np.asarray(inputs["mp1_b"], np.float32)[None, :], (128, 1))),
        "b2m": np.ascontiguousarray(
            np.tile(np.asarray(inputs["mp2_b"], np.float32)[None, :], (128, 1))),
        "b3m": np.ascontiguousarray(
            np.tile(np.asarray(inputs["ff2_b"], np.float32)[None, :], (128, 1))),
        "iota": _bf(iota),
        "identb": _bf(ident),
        "identf": ident,
    }

    per_core = []
    for c in range(NC):
        m = owner_s == c
        e_ids = order[m]
        dp = dest[m]
        eT = np.zeros((64, L), np.float32)
        eT[:, dp] = ef[e_ids].T
        edc = np.full(L, -1.0, np.float32)
        edc[dp] = wl[e_ids]
        sidx = np.zeros(L, np.int16)
        sidx[dp] = (src[e_ids] - halfe[e_ids] * HALF).astype(np.int16)
        didx = np.zeros(L, np.int16)
        didx[dp] = (dst[e_ids] - c * C).astype(np.int16)

        # wrap layouts
        edc_cw = np.ascontiguousarray(edc.reshape(L // 128, 128).T)     # [128, L/128]
        s_wr = np.ascontiguousarray(
            np.tile(sidx.reshape(L // 16, 16).T, (8, 1)))               # [128, L/16]
        d_wr = np.ascontiguousarray(
            np.tile(didx.reshape(L // 16, 16).T, (8, 1)))               # [128, L/16]

        per_core.append({
            "eT": _bf(eT),
            "edc": _bf(edc_cw),
            "srcw": s_wr,
            "xdw": d_wr,
            "xoT": _bf(xpad[c * C:(c + 1) * C].T),
        })
    meta = dict(L=L, L0p=L0p, chunk_meta=chunk_meta)
    return shared, per_core, meta


def _build(L, L0p, chunk_meta):
    """Build the SPMD Bass program (identical for all 8 cores)."""
    nc = bacc.Bacc("TRN2", target_bir_lowering=False, debug=False, num_devices=NC,
                   num_swdge_queues=4,
                   dynamic_dma_scratch_size=int(os.environ.get("KERNEL_DDS", "16384")))
    GELU = (mybir.ActivationFunctionType.Identity
            if os.environ.get("KERNEL_SIM_IDENTITY") == "1"
            else mybir.ActivationFunctionType.Gelu_apprx_tanh)
    EQ = mybir.AluOpType.is_equal

    # I/O
    t_xT = nc.dram_tensor("xT", [128, NPAD], BF, kind="ExternalInput")
    t_xoT = nc.dram_tensor("xoT", [128, C], BF, kind="ExternalInput")
    t_eT = nc.dram_tensor("eT", [64, L], BF, kind="ExternalInput")
    t_edc = nc.dram_tensor("edc", [128, L // 128], BF, kind="ExternalInput")
    t_srcw = nc.dram_tensor("srcw", [128, L // 16], I16, kind="ExternalInput")
    t_xdw = nc.dram_tensor("xdw", [128, L // 16], I16, kind="ExternalInput")
    wts = {}
    for nm, shape, dt in [
        ("W1", [128, 128], BF), ("Ws1", [128, 128], BF), ("Wd1", [128, 128], BF),
        ("We1", [64, 128], BF), ("Ws2", [128, 128], BF), ("Wd2", [128, 128], BF),
        ("We2", [64, 128], BF), ("W3", [128, 128], BF),
        ("b1c", [128, 1], F32), ("b1m", [128, 128], F32), ("b2m", [128, 128], F32),
        ("b3m", [128, 128], F32), ("iota", [128, 128], BF),
        ("identb", [128, 128], BF), ("identf", [128, 128], F32),
    ]:
        wts[nm] = nc.dram_tensor(nm, shape, dt, kind="ExternalInput")
    t_out = nc.dram_tensor("out", [C, 128], F32, kind="ExternalOutput")

    NOPS = L // GOP
    NCH = L // 128

    with tile.TileContext(nc) as tc:
        with (
            tc.tile_pool(name="persist", bufs=1) as pp,
            tc.tile_pool(name="dram", bufs=1, space="DRAM") as dram,
        ):
            # persistent SBUF state
            wt = {}
            for nm in ["W1", "Ws1", "Wd1", "We1", "Ws2", "Wd2", "We2", "W3",
                       "b1c", "b1m", "b2m", "b3m", "iota", "identb", "identf"]:
                shape = wts[nm].shape
                dt = {"b1c": F32, "b1m": F32, "b2m": F32, "b3m": F32,
                      "identf": F32}.get(nm, BF)
                wt[nm] = pp.tile(list(shape), dt, tag=f"w_{nm}", name=f"w_{nm}")
                nc.sync.dma_start(out=wt[nm][:], in_=wts[nm][:])
            edc_t = pp.tile([128, NCH], BF, tag="edc")
            nc.sync.dma_start(out=edc_t[:], in_=t_edc[:])
            srcw_t = pp.tile([128, L // 16], I16, tag="srcw")
            nc.sync.dma_start(out=srcw_t[:], in_=t_srcw[:])
            xdw_t = pp.tile([128, L // 16], I16, tag="xdw")
            nc.sync.dma_start(out=xdw_t[:], in_=t_xdw[:])
            h_own = pp.tile([128, C], BF, tag="h_own")      # node-major own windows
            agg_sb = pp.tile([128, C], F32, tag="agg_sb")   # per-window agg (half 0)

            # internal DRAM
            xs_d = [dram.tile([NPAD, 128], BF, tag=f"xs{l}", name=f"xs{l}") for l in range(2)]
            xd_d = [dram.tile([C, 128], BF, tag=f"xd{l}", name=f"xd{l}") for l in range(2)]
            ag_in = dram.tile([C, 128], BF, tag="ag_in")
            ag_out = dram.tile([NPAD, 128], BF, tag="ag_out", addr_space="Shared")

            r4096 = nc.gpsimd.to_reg(GOP)

            # ---------------- dense phase ----------------
            def dense_full(layer):
                """xs[layer] for all NPAD nodes."""
                Ws = wt["Ws1"] if layer == 0 else wt["Ws2"]
                with (
                    tc.tile_pool(name=f"dA{layer}", bufs=3) as dp,
                    tc.tile_pool(name=f"dAp{layer}", bufs=2, space="PSUM") as dq,
                ):
                    for g in range(NPAD // 512):
                        hT = dp.tile([128, 512], BF, tag="hT")
                        if layer == 0:
                            xt = dp.tile([128, 512], BF, tag="xt")
                            nc.sync.dma_start(
                                out=xt[:], in_=t_xT[:, g * 512:(g + 1) * 512])
                            ps = dq.tile([128, 512], F32, tag="ps")
                            nc.tensor.matmul(out=ps[:], lhsT=wt["W1"][:],
                                             rhs=xt[:], start=True, stop=True)
                            nc.scalar.activation(out=hT[:], in_=ps[:], func=GELU,
                                                 bias=wt["b1c"][:])
                        else:
                            nc.sync.dma_start(
                                out=hT[:],
                                in_=ag_out[g * 512:(g + 1) * 512, :],
                                transpose=True)
                        for j in range(4):
                            n0 = g * 512 + j * 128
                            px = dq.tile([128, 128], F32, tag="px")
                            nc.tensor.matmul(out=px[:],
                                             lhsT=hT[:, j * 128:(j + 1) * 128],
                                             rhs=Ws[:], start=True, stop=True)
                            xs_sb = dp.tile([128, 128], BF, tag="xs_sb")
                            nc.vector.tensor_copy(out=xs_sb[:], in_=px[:])
                            nc.sync.dma_start(out=xs_d[layer][n0:n0 + 128, :],
                                              in_=xs_sb[:])

            def dense_own_l0():
                """h0 own (node-major) + xd0+b for own nodes, from x_own_T."""
                with (
                    tc.tile_pool(name="dB", bufs=3) as dp,
                    tc.tile_pool(name="dBp", bufs=2, space="PSUM") as dq,
                ):
                    ngrp = (C + 511) // 512
                    for g in range(ngrp):
                        c0 = g * 512
                        cn = min(512, C - c0)
                        xt = dp.tile([128, 512], BF, tag="xt")
                        nc.sync.dma_start(out=xt[:, :cn], in_=t_xoT[:, c0:c0 + cn])
                        ps = dq.tile([128, 512], F32, tag="ps")
                        nc.tensor.matmul(out=ps[:, :cn], lhsT=wt["W1"][:],
                                         rhs=xt[:, :cn], start=True, stop=True)
                        hT = dp.tile([128, 512], BF, tag="hT")
                        nc.scalar.activation(out=hT[:, :cn], in_=ps[:, :cn],
                                             func=GELU, bias=wt["b1c"][:])
                        for j in range(cn // 128):
                            lw = c0 + j * 128
                            sl = hT[:, j * 128:(j + 1) * 128]
                            pn = dq.tile([128, 128], F32, tag="pn")
                            nc.tensor.matmul(out=pn[:], lhsT=sl, rhs=wt["identb"][:],
                                             start=True, stop=True)
                            nc.vector.tensor_copy(out=h_own[:, lw:lw + 128], in_=pn[:])
                            pd = dq.tile([128, 128], F32, tag="pd")
                            nc.tensor.matmul(out=pd[:], lhsT=sl, rhs=wt["Wd1"][:],
                                             start=True, stop=True)
                            xd_sb = dp.tile([128, 128], BF, tag="xd_sb")
                            nc.vector.tensor_add(out=xd_sb[:], in0=pd[:],
                                                 in1=wt["b1m"][:])
                            nc.sync.dma_start(out=xd_d[0][lw:lw + 128, :],
                                              in_=xd_sb[:])

            def dense_own_l1():
                """xd1+b for own nodes from h_own (h1, node-major in SBUF)."""
                with (
                    tc.tile_pool(name="dC", bufs=3) as dp,
                    tc.tile_pool(name="dCp", bufs=2, space="PSUM") as dq,
                ):
                    for w in range(WCNT):
                        ws = slice(w * 128, (w + 1) * 128)
                        pt = dq.tile([128, 128], BF, tag="pt")
                        nc.tensor.transpose(out=pt[:], in_=h_own[:, ws],
                                            identity=wt["identb"][:])
                        hT = dp.tile([128, 128], BF, tag="hT")
                        nc.vector.tensor_copy(out=hT[:], in_=pt[:])
                        pd = dq.tile([128, 128], F32, tag="pd")
                        nc.tensor.matmul(out=pd[:], lhsT=hT[:], rhs=wt["Wd2"][:],
                                         start=True, stop=True)
                        xd_sb = dp.tile([128, 128], BF, tag="xd_sb")
                        nc.vector.tensor_add(out=xd_sb[:], in0=pd[:], in1=wt["b2m"][:])
                        nc.sync.dma_start(out=xd_d[1][w * 128:(w + 1) * 128, :],
                                          in_=xd_sb[:])

            # ---------------- edge phase ----------------
            def edge_phase(layer):
                We = wt["We1"] if layer == 0 else wt["We2"]
                with (
                    tc.tile_pool(name=f"eS{layer}", bufs=2) as ep,
                    tc.tile_pool(name=f"eG{layer}", bufs=2) as gp,
                    tc.tile_pool(name=f"eP{layer}", bufs=2, space="PSUM") as qp,
                    tc.tile_pool(name=f"eA{layer}", bufs=2, space="PSUM") as ap_,
                    tc.tile_pool(name=f"eF{layer}", bufs=2) as fp,
                ):
                    active_agg = [None]

                    def finalize(h, w, agg_ps):
                        ws = slice(w * 128, (w + 1) * 128)
                        if h == 0:
                            nc.vector.tensor_copy(out=agg_sb[:, ws], in_=agg_ps[:])
                            return
                        t1 = fp.tile([128, 128], F32, tag="f1")
                        nc.vector.tensor_add(out=t1[:], in0=agg_ps[:],
                                             in1=agg_sb[:, ws])
                        if layer == 0:
                            h1w = fp.tile([128, 128], BF, tag="fh")
                            nc.vector.tensor_add(out=h1w[:], in0=t1[:],
                                                 in1=h_own[:, ws])
                            nc.vector.tensor_copy(out=h_own[:, ws], in_=h1w[:])
                            nc.sync.dma_start(out=ag_in[w * 128:(w + 1) * 128, :],
                                              in_=h1w[:])
                        else:
                            h2w = fp.tile([128, 128], F32, tag="fh2")
                            nc.vector.tensor_add(out=h2w[:], in0=t1[:],
                                                 in1=h_own[:, ws])
                            pt = ap_.tile([128, 128], F32, tag="tp", bufs=1)
                            nc.tensor.transpose(out=pt[:], in_=h2w[:],
                                                identity=wt["identf"][:])
                            h2T = fp.tile([128, 128], BF, tag="h2T")
                            nc.vector.tensor_copy(out=h2T[:], in_=pt[:])
                            po = ap_.tile([128, 128], F32, tag="po", bufs=1)
                            nc.tensor.matmul(out=po[:], lhsT=h2T[:], rhs=wt["W3"][:],
                                             start=True, stop=True)
                            osb = fp.tile([128, 128], F32, tag="osb")
                            nc.vector.tensor_add(out=osb[:], in0=po[:],
                                                 in1=wt["b3m"][:])
                            nc.sync.dma_start(out=t_out[w * 128:(w + 1) * 128, :],
                                              in_=osb[:])

                    for op_i in range(NOPS):
                        e0 = op_i * GOP
                        cc0 = e0 // 128
                        nreal = sum(1 for k in range(GOP // 128) if chunk_meta[cc0 + k][4])
                        if nreal == 0:
                            continue
                        half = 0 if e0 < L0p else 1
                        src_ap = xs_d[layer][half * HALF:(half + 1) * HALF, :]
                        xsg = gp.tile([128, GOP // 128, 128], BF, tag="xsg")
                        nc.gpsimd.dma_gather(
                            xsg[:], src_ap, srcw_t[:, e0 // 16:(e0 + GOP) // 16],
                            GOP, r4096, 128, elem_step=128,
                            queue_num=(2 * op_i) % 4)
                        xdg = gp.tile([128, GOP // 128, 128], BF, tag="xdg")
                        nc.gpsimd.dma_gather(
                            xdg[:], xd_d[layer][:], xdw_t[:, e0 // 16:(e0 + GOP) // 16],
                            GOP, r4096, 128, elem_step=128,
                            queue_num=(2 * op_i + 1) % 4)
                        eTt = ep.tile([64, GOP], BF, tag="eTt")
                        nc.sync.dma_start(out=eTt[:], in_=t_eT[:, e0:e0 + GOP])

                        for sb0 in range(0, nreal, SBC):
                            nb = min(SBC, nreal - sb0)
                            psq = qp.tile([128, SBC, 128], F32, tag="psq")
                            tsx = ep.tile([128, SBC, 128], BF, tag="tsx")
                            nc.vector.tensor_add(out=tsx[:, :nb, :],
                                                 in0=xsg[:, sb0:sb0 + nb, :],
                                                 in1=xdg[:, sb0:sb0 + nb, :])
                            for j in range(nb):
                                cl = sb0 + j
                                nc.tensor.matmul(
                                    out=psq[:, j, :],
                                    lhsT=eTt[:, cl * 128:(cl + 1) * 128],
                                    rhs=We[:], start=True, stop=False)
                                nc.tensor.matmul(
                                    out=psq[:, j, :], lhsT=wt["identb"][:],
                                    rhs=tsx[:, j, :], start=False, stop=True)
                            mt = ep.tile([128, SBC, 128], BF, tag="mt")
                            nc.scalar.activation(out=mt[:, :nb, :],
                                                 in_=psq[:, :nb, :], func=GELU)
                            P4 = ep.tile([128, SBC, 128], BF, tag="P4")
                            nc.vector.tensor_tensor(
                                out=P4[:, :nb, :],
                                in0=edc_t[:, cc0 + sb0:cc0 + sb0 + nb]
                                    .to_broadcast([128, nb, 128]),
                                in1=wt["iota"][:, None, :]
                                    .to_broadcast([128, nb, 128]),
                                op=EQ)
                            for j in range(nb):
                                cc = cc0 + sb0 + j
                                w, first, last, hh, real = chunk_meta[cc]
                                assert real
                                if first:
                                    active_agg[0] = ap_.tile([128, 128], F32, tag="agg", name="agg_ps")
                                nc.tensor.matmul(out=active_agg[0][:],
                                                 lhsT=P4[:, j, :], rhs=mt[:, j, :],
                                                 start=first, stop=last)
                                if last:
                                    finalize(hh, w, active_agg[0])

            # ---------------- program ----------------
            phases = int(os.environ.get("KERNEL_PHASES", "5"))
            dense_full(0)
            dense_own_l0()
            if phases >= 2:
                edge_phase(0)
            if phases >= 3:
                nc.gpsimd.collective_compute(
                    "AllGather", mybir.AluOpType.bypass,
                    replica_groups=[list(range(NC))],
                    ins=[ag_in[:].opt()], outs=[ag_out[:].opt()])
            if phases >= 4:
                dense_full(1)
                dense_own_l1()
            if phases >= 5:
                edge_phase(1)
            else:
                with tc.tile_pool(name="dbg", bufs=2) as dbp:
                    for w in range(WCNT):
                        dsb = dbp.tile([128, 128], F32, tag="dsb")
                        nc.vector.tensor_copy(out=dsb[:], in_=h_own[:, w * 128:(w + 1) * 128])
                        nc.sync.dma_start(out=t_out[w * 128:(w + 1) * 128, :], in_=dsb[:])

    nc.finalize()
    return nc


_CACHE = {}


def _get_program(L, L0p, chunk_meta):
    key = (L, L0p, tuple(m[:4] for m in chunk_meta))
    if key not in _CACHE:
        _CACHE[key] = _build(L, L0p, chunk_meta)
    return _CACHE[key]


def kernel(**inputs):
    shared, per_core, meta = _prep(inputs)
    nc = _get_program(meta["L"], meta["L0p"], meta["chunk_meta"])
    in_maps = []
    for c in range(NC):
        m = dict(shared)
        m.update(per_core[c])
        in_maps.append(m)
    trace = os.environ.get("KERNEL_TRACE", "0") == "1"
    kw = {}
    if trace:
        kw = dict(trace=True, trace_kwargs={"title": "gnn_mp"})
    res = run_bass_kernel_spmd(nc, in_maps, core_ids=list(range(NC)), **kw)
    if trace and res.exec_time_ns is not None:
        print(f"HW exec time: {res.exec_time_ns} ns")
        if res.instructions_and_trace:
            print("trace:", res.instructions_and_trace[1])
    out = np.concatenate([res.results[c]["out"] for c in range(NC)], axis=0)
    return np.ascontiguousarray(out[:N]).astype(np.float32)
